# revision 54
# baseline (speedup 1.0000x reference)
"""Multi-head self-attention Trainium2 kernel.

Sharding: 8 cores = 2 batches x 4 head-groups. Core c handles batch c//4 and
heads [4g, 4g+4) where g = c%4 (dims [256g, 256g+256) of the 1024 model dim).

Per-core device program (all matmul operands bf16, fp32 PSUM accumulation).
The scalar engine's exp stream (128 ops x ~1.04us over the 16.8M attention
scores) is the throughput floor, so the whole schedule is built to keep it
dense from ~10us (first scores) to the end:
  - QT/KT projections computed transposed: QT[d, t] = Wq_g @ x_b^T (+bias; Q
    also scaled by 1/sqrt(64)), d-major weight halves DMA'd first so the
    first score tile exists as early as the DMA pipe allows. K stored packed
    [hh*64+dh, t]; score matmuls contract over a 64-partition slice. V in
    natural [token, dim] layout.
  - Attention runs as two decoupled software-pipelined streams:
      ST/exp stream: S^T tiles [128 keys, 2 heads, 512 q] -> one wide exp per
      tile on the scalar engine into a bf16 SBUF ring (EX_WINDOW tiles). The
      2-deep score PSUM ring means ST(e+2) waits exp(e), so STs are placed in
      the PE stream by a credit scheduler at ~1us spacing, interleaved inside
      projection groups (availability-greedy order over (unit, key-tile)).
      PV stream (trails by PV_LAG): flipped PV -- ctx[q, dh] += exp(S)^T.T @ V
      with q on partitions, so each matmul charges 64 output rows instead of
      512. Softmax denominators accumulate via rhs=ones [128,1] matmuls.
      PSUM accumulator banks hold 8 groups each: start/stop are asserted once
      per bank per round (start zeroes the whole 2KB zero region).
  - Unit epilogue: one PSUM->SBUF copy releases the ctx bank for the next
    unit (~1us earlier), then DVE reciprocal + per-(head,qt) scales, PE
    transpose (via identity) back to ctx^T[(hh,dh), q]; out-projection groups
    drip into the PE gaps at unit seams; the final unit runs a per-qt
    pipelined tail with evictions split across scalar+vector engines.
  - Out partials stored bf16; host sums the 4 group partials per batch and
    adds b_out + b_v @ W_out^T (V-bias commutes through softmax).
"""

import numpy as np
import ml_dtypes

import concourse.bacc as bacc
import concourse.mybir as mybir
from concourse.tile import TileContext
from concourse.bass_utils import run_bass_kernel_spmd
from concourse.masks import make_identity

AF = mybir.ActivationFunctionType
ALU = mybir.AluOpType
F32 = mybir.dt.float32
BF16 = mybir.dt.bfloat16
NP_BF16 = ml_dtypes.bfloat16

B, S, D, H, DH = 2, 2048, 1024, 16, 64
DG = 256          # dims per head-group (4 heads)
TC = 512          # token / query chunk
NTC = S // TC     # 4
NTT = S // 128    # 16 token tiles
NKT = S // 128    # 16 key tiles

EX_WINDOW = 48    # bf16 S^T-exp tiles buffered ahead of the PV stream
PV_LAG = 4        # PV stream trails the ST/exp stream by >= this many steps
N_WARM = 6        # PE p-state warm-up matmuls

# bit pattern of two bf16 1.0s packed in a float32 (for memset on bf16 tiles)
_ONES_BF16_F32 = float(
    np.frombuffer(np.array([0x3F803F80], np.uint32).tobytes(), np.float32)[0]
)

_NC_CACHE = None


def _build_nc():
    nc = bacc.Bacc("TRN2", target_bir_lowering=False, debug=False)

    xT = nc.dram_tensor("xT", [D, S], BF16, kind="ExternalInput")
    wq0 = nc.dram_tensor("wq0", [128, 8, 128], BF16, kind="ExternalInput")
    wq1 = nc.dram_tensor("wq1", [128, 8, 128], BF16, kind="ExternalInput")
    wk0 = nc.dram_tensor("wk0", [128, 8, 128], BF16, kind="ExternalInput")
    wk1 = nc.dram_tensor("wk1", [128, 8, 128], BF16, kind="ExternalInput")
    wv = nc.dram_tensor("wvT", [D, DG], BF16, kind="ExternalInput")
    wo = nc.dram_tensor("woT", [DG, D], BF16, kind="ExternalInput")
    bq = nc.dram_tensor("bq", [2, 128], F32, kind="ExternalInput")
    bk = nc.dram_tensor("bk", [2, 128], F32, kind="ExternalInput")
    out = nc.dram_tensor("out", [S, D], BF16, kind="ExternalOutput")

    with TileContext(nc) as tc:
        with (
            tc.tile_pool(name="const", bufs=1) as constp,
            tc.tile_pool(name="xt", bufs=3) as xtp,
            tc.tile_pool(name="expst", bufs=EX_WINDOW) as expp,
            tc.tile_pool(name="ctxn", bufs=2) as ctxnp,
            tc.tile_pool(name="small", bufs=2) as smallp,
            tc.tile_pool(name="outp", bufs=4) as outp,
            tc.tile_pool(name="pp_ps", bufs=2, space="PSUM") as pp,
            tc.tile_pool(name="s_ps", bufs=2, space="PSUM") as sps,
            tc.tile_pool(name="ctx_ps", bufs=1, space="PSUM") as ctxps,
            tc.tile_pool(name="dn_ps", bufs=1, space="PSUM") as dnp,
        ):
            # ---- persistent tiles ----
            # Q/K weights are d-major: the d0 halves arrive first so the
            # first Q/K projection groups (which gate the scalar engine's
            # entire exp stream) start as early as the DMA pipe allows
            wq_s = constp.tile([128, 2, 8, 128], BF16)
            wk_s = constp.tile([128, 2, 8, 128], BF16)
            wv_s = constp.tile([128, 8, DG], BF16)
            bq_s = constp.tile([128, 2], F32)
            bk_s = constp.tile([128, 2], F32)
            xTr = xT[:, :].rearrange("(k p) t -> p k t", p=128)
            xt0 = xtp.tile([128, 8, TC], BF16, name="xt0", tag="xt")
            nc.sync.dma_start(out=wq_s[:, 0, :, :], in_=wq0[:, :, :])
            nc.sync.dma_start(out=xt0[:, 0:4, :], in_=xTr[:, 0:4, 0:TC])
            nc.sync.dma_start(out=wk_s[:, 0, :, :], in_=wk0[:, :, :])
            nc.sync.dma_start(out=xt0[:, 4:8, :], in_=xTr[:, 4:8, 0:TC])
            nc.sync.dma_start(out=bq_s, in_=bq[:, :].rearrange("t p -> p t"))
            nc.sync.dma_start(out=bk_s, in_=bk[:, :].rearrange("t p -> p t"))
            nc.sync.dma_start(out=wq_s[:, 1, :, :], in_=wq1[:, :, :])
            nc.sync.dma_start(out=wk_s[:, 1, :, :], in_=wk1[:, :, :])
            nc.sync.dma_start(out=wv_s,
                              in_=wv[:, :].rearrange("(k p) m -> p k m", p=128))

            QT_s = constp.tile([128, 2, S], BF16)
            KT_s = constp.tile([128, 2, S], BF16)
            ctxT_s = constp.tile([128, 2, S], BF16)
            Vg_s = constp.tile([128, NTT, 4, DH], BF16)
            wo_s = constp.tile([128, 2, D], BF16)
            ident = constp.tile([128, 128], BF16)
            make_identity(nc, ident)
            ones = constp.tile([128, 2], BF16)
            nc.vector.memset(ones.bitcast(F32), _ONES_BF16_F32)

            # warm the PE clock (p-state ramp) with dummy matmuls while the
            # initial DMAs stream in, so real matmuls start at full rate
            warm = constp.tile([128, TC], BF16)
            nc.vector.memset(warm.bitcast(F32), _ONES_BF16_F32)
            wps = pp.tile([128, TC], F32, tag="p", name="wps")
            for _ in range(N_WARM):
                nc.tensor.matmul(wps, lhsT=warm[:, 0:128], rhs=warm,
                                 start=True, stop=True)

            # ---- projection group emitters (q/k transposed, v natural) ----
            # note_pe is a drip hook called after each matmul with its PE cost
            def emit_q_group(d, tci, xt, note_pe):
                tsl = slice(tci * TC, (tci + 1) * TC)
                dsl = slice(d * 128, (d + 1) * 128)
                psq = pp.tile([128, TC], F32, tag="p", name="psq")
                for k in range(8):
                    nc.tensor.matmul(psq, lhsT=wq_s[:, d, k, :], rhs=xt[:, k, :],
                                     start=(k == 0), stop=(k == 7))
                    note_pe(213)
                nc.vector.tensor_scalar(QT_s[:, d, tsl], psq,
                                        scalar1=bq_s[:, d:d + 1], scalar2=0.125,
                                        op0=ALU.add, op1=ALU.mult)

            def emit_k_group(d, tci, xt, note_pe):
                tsl = slice(tci * TC, (tci + 1) * TC)
                dsl = slice(d * 128, (d + 1) * 128)
                psk = pp.tile([128, TC], F32, tag="p", name="psk")
                for k in range(8):
                    nc.tensor.matmul(psk, lhsT=wk_s[:, d, k, :], rhs=xt[:, k, :],
                                     start=(k == 0), stop=(k == 7))
                    note_pe(213)
                nc.vector.tensor_scalar(KT_s[:, d, tsl], psk,
                                        scalar1=bk_s[:, d:d + 1],
                                        scalar2=None, op0=ALU.add)

            def emit_v_group(tci, tt, xt, note_pe):
                ti = tci * 4 + tt
                psv = pp.tile([128, DG], F32, tag="p", name="psv")
                for k in range(8):
                    nc.tensor.matmul(psv, lhsT=xt[:, k, tt * 128:(tt + 1) * 128],
                                     rhs=wv_s[:, k, :], start=(k == 0), stop=(k == 7))
                    note_pe(107)
                nc.vector.tensor_copy(Vg_s[:, ti, :, :], psv)
                ti_done.add(ti)

            # ---- attention streams ----
            # units u = (qc, d); steps (u, kt). The ST/exp stream leads (bf16
            # ex ring), the PV stream trails and accumulates flipped ctx.
            units = [(qc, d) for qc in range(NTC) for d in range(2)]
            n_steps = len(units) * NKT
            ex_of = {}
            ex_idx = {}       # (u, kt) -> ST emission counter at emission
            emitted = set()
            cps_of = {}
            dn_of = {}
            drip = []
            pending_tr = []   # deferred (ready_at_pv, u, ctx_n) transposes
            q_done = set()    # (qc, d) Q groups evicted
            k_done = set()    # (d, kchunk) K groups evicted
            ti_done = set()   # V token-tiles evicted

            def emit_st(u, kt):
                qc, d = units[u]
                qsl = slice(qc * TC, (qc + 1) * TC)
                ksl = slice(kt * 128, (kt + 1) * 128)
                sp = sps.tile([128, 2, TC], F32, tag="s")
                for hh in range(2):
                    p0 = 64 * hh
                    nc.tensor.matmul(sp[:, hh, :],
                                     lhsT=KT_s[p0:p0 + 64, d, ksl],
                                     rhs=QT_s[p0:p0 + 64, d, qsl],
                                     start=True, stop=True)
                ex = expp.tile([128, 2, TC], BF16)
                nc.scalar.activation(ex, sp, AF.Exp)
                ex_of[(u, kt)] = ex

            def next_st():
                # earliest (u, kt) in PV consumption order whose Q chunk and
                # K chunk are both evicted -- front-fills the PV frontier
                for u in range(len(units)):
                    qc, d = units[u]
                    if (qc, d) not in q_done:
                        continue
                    for kt in range(NKT):
                        if (u, kt) not in emitted and (d, kt // 4) in k_done:
                            return (u, kt)
                return None

            def emit_pv(i):
                u, kt = divmod(i, NKT)
                qc, d = units[u]
                ex = ex_of.pop((u, kt))
                if kt == 0:
                    cps_of[u] = ctxps.tile([128, 8, DH], F32, tag="ctx",
                                           name=f"cps{u}")
                    dn_of[u] = dnp.tile([128, 8], F32, tag="dn", name=f"dn{u}")
                cps, dn = cps_of[u], dn_of[u]
                first, last = (kt == 0), (kt == NKT - 1)
                # PSUM "start" zeroes the whole 2KB bank (zero region), so a
                # bank holding 8 accumulation groups must be started exactly
                # once (first group's first matmul) and stopped once (last
                # group's last matmul); later groups' first writes land on
                # pending-zero bytes and start cleanly.
                for hh in range(2):
                    h = 2 * d + hh
                    for qt in range(4):
                        j = hh * 4 + qt
                        lhs = ex[:, hh, qt * 128:(qt + 1) * 128]
                        nc.tensor.matmul(cps[:, j, :], lhsT=lhs,
                                         rhs=Vg_s[:, kt, h, :],
                                         start=(first and j == 0),
                                         stop=(last and j == 7))
                        nc.tensor.matmul(dn[:, j:j + 1], lhsT=lhs,
                                         rhs=ones[:, 0:1],
                                         start=(first and j == 0),
                                         stop=(last and j == 7))
                if last and u == len(units) - 1:
                    emit_last_unit_tail(u)
                elif last:
                    # one fast PSUM->SBUF copy releases the ctx bank (~0.7us)
                    # so the next unit's first PV isn't stalled behind the 8
                    # normalize muls (~1.7us); the muls then read the copy
                    cps, dn = cps_of.pop(u), dn_of.pop(u)
                    rec = smallp.tile([128, 8], F32, tag="rec")
                    nc.vector.reciprocal(rec, dn)
                    ctxf = ctxnp.tile([128, 8, DH], F32, tag="ctxf")
                    nc.vector.tensor_copy(ctxf, cps)
                    ctx_n = ctxnp.tile([128, 4, 2, DH], BF16)
                    for qt in range(4):
                        for hh in range(2):
                            j = hh * 4 + qt
                            nc.vector.tensor_scalar(ctx_n[:, qt, hh, :],
                                                    ctxf[:, j, :],
                                                    scalar1=rec[:, j:j + 1],
                                                    scalar2=None, op0=ALU.mult)
                    pending_tr.append((pv_i + 2, u, ctx_n))

            def emit_transpose_stage(u, ctx_n):
                qc, d = units[u]
                psT = pp.tile([128, 4, 128], BF16, tag="p", name="psT")
                qsl = slice(qc * TC, (qc + 1) * TC)
                for qt in range(4):
                    nc.tensor.transpose(psT[:, qt, :], ctx_n[:, qt, :, :], ident)
                nc.vector.tensor_copy(ctxT_s[:, d, qsl], psT)
                if d == 1:
                    drip.extend(range(qc * 4, (qc + 1) * 4))

            def emit_last_unit_tail(u):
                # tail: per-qt chains with the DVE spine front-loaded
                # (rec + both muls per qt, transpose, psT evict, then that
                # qt's out-projection immediately); output evictions split
                # between the now-idle scalar engine and DVE
                qc, d = units[u]
                cps, dn = cps_of.pop(u), dn_of.pop(u)
                rec = smallp.tile([128, 8], F32, tag="rec")
                nc.vector.reciprocal(rec, dn)
                ctx_n = ctxnp.tile([128, 4, 2, DH], BF16)
                for qt in range(4):
                    for hh in range(2):
                        j = hh * 4 + qt
                        nc.vector.tensor_scalar(ctx_n[:, qt, hh, :],
                                                cps[:, j, :],
                                                scalar1=rec[:, j:j + 1],
                                                scalar2=None, op0=ALU.mult)
                    psT = pp.tile([128, 128], BF16, tag="p", name="psTq")
                    nc.tensor.transpose(psT, ctx_n[:, qt, :, :], ident)
                    tt = qc * 4 + qt
                    nc.vector.tensor_copy(
                        ctxT_s[:, d, tt * 128:(tt + 1) * 128], psT)
                    ev = (nc.scalar, nc.scalar) if qt == 0 else \
                        (nc.scalar, nc.vector)
                    emit_d_group(tt, evict_engines=ev,
                                 po_tag=("s", "p")[qt % 2],
                                 split_store=(qt == 3))

            def emit_d_group(tt, evict_engines=None, po_tag="p",
                             split_store=False):
                # one token-tile: both 512-wide halves of the out row block,
                # merged into a single bf16 store (split per-half for the
                # final tile so its first DMA overlaps the second eviction)
                psl = slice(tt * 128, (tt + 1) * 128)
                ot = outp.tile([128, 2, TC], BF16)
                for oc in range(2):
                    osl = slice(oc * TC, (oc + 1) * TC)
                    pool = pp if po_tag == "p" else sps
                    po = pool.tile([128, TC], F32, tag=po_tag, name="po")
                    for dd in range(2):
                        nc.tensor.matmul(po, lhsT=ctxT_s[:, dd, psl],
                                         rhs=wo_s[:, dd, osl],
                                         start=(dd == 0), stop=(dd == 1))
                    eng = (evict_engines or (nc.vector, nc.vector))[oc]
                    if eng is nc.scalar:
                        eng.copy(ot[:, oc, :], po)
                    else:
                        eng.tensor_copy(ot[:, oc, :], po)
                    if split_store:
                        nc.sync.dma_start(out=out[psl, osl], in_=ot[:, oc, :])
                if not split_store:
                    nc.sync.dma_start(out=out[psl, :], in_=ot)

            # ---- phase B: projections with interleaved ST/exp/PV streams ----
            # Credit-based drip: the score PSUM ring is only 2 deep, so
            # ST(e+2) waits for exp(e); STs must be spaced ~1.04us apart in
            # the PE stream (the exp duration) or they head-of-line block it.
            # Each emitted PE instruction adds its cost to `credit`; an ST is
            # dripped once enough other work sits between it and the last one.
            st_n = 0          # STs emitted
            pv_i = 0          # PV steps emitted (consumption order)
            credit = 10 ** 9

            def pv_ready():
                key = divmod(pv_i, NKT)
                if key not in ex_of or key[1] not in ti_done:
                    return False
                # PV must trail its exp far enough not to block the PE stream
                return ex_idx[key] <= st_n - PV_LAG or st_n == n_steps

            def note_pe(ns):
                nonlocal credit, st_n, pv_i
                credit += ns
                progress = True
                while progress:
                    progress = False
                    nxt = next_st()
                    if credit >= 500 and nxt is not None \
                            and st_n - pv_i < EX_WINDOW - 2:
                        emit_st(*nxt)
                        emitted.add(nxt)
                        ex_idx[nxt] = st_n
                        st_n += 1
                        credit = 0
                        progress = True
                    if pv_i < n_steps and pv_ready():
                        emit_pv(pv_i)
                        pv_i += 1
                        credit += 240
                        progress = True
                    if pending_tr and pending_tr[0][0] <= pv_i:
                        _, uu, cn = pending_tr.pop(0)
                        emit_transpose_stage(uu, cn)
                        credit += 213
                        progress = True

            # flat projection order: chunk 1's K groups are pulled between
            # chunk 0's V groups so the ST stream never runs out of
            # availability while the scalar engine still has backlog
            xts = [xt0]
            for tci in range(1, NTC):
                xt = xtp.tile([128, 8, TC], BF16, tag="xt", name=f"xt{tci}")
                nc.sync.dma_start(out=xt, in_=xTr[:, :, tci * TC:(tci + 1) * TC])
                if tci == 1:
                    nc.sync.dma_start(
                        out=wo_s, in_=wo[:, :].rearrange("(k p) m -> p k m", p=128))
                xts.append(xt)

            # chunk 0's first Q/K groups interleave their matmul halves
            # with the DMA half-arrivals (wq_a, xt_a, wk_a, wq_b, ...)
            psq0 = pp.tile([128, TC], F32, tag="p", name="psq0")
            psk0 = pp.tile([128, TC], F32, tag="p", name="psk0")
            for k in range(4):
                nc.tensor.matmul(psq0, lhsT=wq_s[:, 0, k, :], rhs=xt0[:, k, :],
                                 start=(k == 0), stop=False)
            for k in range(4):
                nc.tensor.matmul(psk0, lhsT=wk_s[:, 0, k, :], rhs=xt0[:, k, :],
                                 start=(k == 0), stop=False)
            for k in range(4, 8):
                nc.tensor.matmul(psq0, lhsT=wq_s[:, 0, k, :], rhs=xt0[:, k, :],
                                 start=False, stop=(k == 7))
            for k in range(4, 8):
                nc.tensor.matmul(psk0, lhsT=wk_s[:, 0, k, :], rhs=xt0[:, k, :],
                                 start=False, stop=(k == 7))
            nc.vector.tensor_scalar(QT_s[:, 0, 0:TC], psq0,
                                    scalar1=bq_s[:, 0:1], scalar2=0.125,
                                    op0=ALU.add, op1=ALU.mult)
            nc.vector.tensor_scalar(KT_s[:, 0, 0:TC], psk0,
                                    scalar1=bk_s[:, 0:1],
                                    scalar2=None, op0=ALU.add)
            q_done.add((0, 0))
            k_done.add((0, 0))
            # seed the exp stream with two back-to-back STs so exp#1 follows
            # exp#0 without a gap (the credit drip would space them apart)
            for _ in range(2):
                nxt = next_st()
                emit_st(*nxt)
                emitted.add(nxt)
                ex_idx[nxt] = st_n
                st_n += 1
            note_pe(0)

            flat = ["0q1", "0k1", "0v0", "1k0", "0v1", "1k1",
                    "0v2", "1q0", "0v3", "1q1", "1v0", "1v1", "1v2", "1v3",
                    "2k0", "2k1", "2q0", "2q1", "2v0", "2v1", "2v2", "2v3",
                    "3k0", "3k1", "3q0", "3q1", "3v0", "3v1", "3v2", "3v3"]
            for g in flat:
                tci, kind, idx = int(g[0]), g[1], int(g[2])
                if kind == "q":
                    emit_q_group(idx, tci, xts[tci], note_pe)
                    q_done.add((tci, idx))
                elif kind == "k":
                    emit_k_group(idx, tci, xts[tci], note_pe)
                    k_done.add((idx, tci))
                else:
                    emit_v_group(tci, idx, xts[tci], note_pe)

            # ---- phase C: attention steady state ----
            # PVs are emitted before STs: a PV's exp is PV_LAG behind and
            # never blocks the in-order PE stream, while an ST waits on the
            # 2-deep score ring (exp(e-2)); this drains the PV backlog faster
            # at the tail
            while pv_i < n_steps:
                did = False
                if pv_ready():
                    emit_pv(pv_i)
                    pv_i += 1
                    did = True
                nxt = next_st()
                if nxt is not None and st_n - pv_i < EX_WINDOW:
                    emit_st(*nxt)
                    emitted.add(nxt)
                    ex_idx[nxt] = st_n
                    st_n += 1
                    did = True
                if not did and pv_i < n_steps:
                    emit_pv(pv_i)
                    pv_i += 1
                if drip and (pv_i % NKT == NKT - 1 or pv_i % 3 == 0
                             or (6 * NKT <= pv_i < n_steps - 8 and
                                 pv_i % 2 == 0)):
                    # out-projection groups preferentially fill the PE bubble
                    # at unit seams; fully drained before the last unit's
                    # final steps so they don't delay the critical tail
                    emit_d_group(drip.pop(0))
                while pending_tr and pending_tr[0][0] <= pv_i:
                    _, uu, cn = pending_tr.pop(0)
                    emit_transpose_stage(uu, cn)
            # tail: alternate evictions across idle engines, po rings from
            # both free PSUM pools to avoid eviction-serialized stalls
            engs = [(nc.gpsimd, nc.vector), (nc.scalar, nc.gpsimd),
                    (nc.vector, nc.scalar)]
            for i, g in enumerate(drip):
                emit_d_group(g, evict_engines=engs[i % 3],
                             po_tag=("p", "s")[i % 2])

    nc.finalize()
    return nc


def get_nc():
    global _NC_CACHE
    if _NC_CACHE is None:
        _NC_CACHE = _build_nc()
    return _NC_CACHE


def make_in_maps(x, W_q, b_q, W_k, b_k, W_v, b_v, W_out, b_out):
    xb = [np.ascontiguousarray(x[b].T).astype(NP_BF16) for b in range(B)]
    in_maps = []
    for c in range(8):
        b, g = divmod(c, 4)
        sl = slice(DG * g, DG * (g + 1))
        wqT = W_q[sl, :].T.astype(NP_BF16)
        wkT = W_k[sl, :].T.astype(NP_BF16)

        def dmajor(wT, dd):
            # [D, 128] slice -> [128 part, 8 kchunk, 128 dim], row-contiguous
            m = wT[:, dd * 128:(dd + 1) * 128].reshape(8, 128, 128)
            return np.ascontiguousarray(m.transpose(1, 0, 2))

        in_maps.append({
            "xT": xb[b],
            "wq0": dmajor(wqT, 0), "wq1": dmajor(wqT, 1),
            "wk0": dmajor(wkT, 0), "wk1": dmajor(wkT, 1),
            "wvT": np.ascontiguousarray(W_v[sl, :].T).astype(NP_BF16),
            "woT": np.ascontiguousarray(W_out[:, sl].T).astype(NP_BF16),
            "bq": b_q[sl].reshape(2, 128).astype(np.float32),
            "bk": b_k[sl].reshape(2, 128).astype(np.float32),
        })
    return in_maps


def combine_outputs(outs, W_out, b_out, b_v):
    host_bias = (b_out + b_v @ W_out.T).astype(np.float32)
    y = np.empty((B, S, D), np.float32)
    for b in range(B):
        y[b] = (outs[4 * b].astype(np.float32)
                + outs[4 * b + 1].astype(np.float32)
                + outs[4 * b + 2].astype(np.float32)
                + outs[4 * b + 3].astype(np.float32))
        y[b] += host_bias
    return y


def kernel(x, W_q, b_q, W_k, b_k, W_v, b_v, W_out, b_out):
    x = np.asarray(x, dtype=np.float32)
    args = [np.asarray(a, dtype=np.float32)
            for a in (W_q, b_q, W_k, b_k, W_v, b_v, W_out, b_out)]
    W_q, b_q, W_k, b_k, W_v, b_v, W_out, b_out = args
    nc = get_nc()
    in_maps = make_in_maps(x, W_q, b_q, W_k, b_k, W_v, b_v, W_out, b_out)
    last_err = None
    for attempt in range(3):
        try:
            res = run_bass_kernel_spmd(nc, in_maps, core_ids=list(range(8)))
            break
        except Exception as e:  # transient device-unrecoverable flakes
            last_err = e
            import time
            time.sleep(10)
    else:
        raise last_err
    outs = [r["out"] for r in res.results]
    return combine_outputs(outs, W_out, b_out, b_v)


# revision 63
# speedup vs baseline: 1.0057x; 1.0057x over previous
"""Multi-head self-attention Trainium2 kernel.

Sharding: 8 cores = 2 batches x 4 head-groups. Core c handles batch c//4 and
heads [4g, 4g+4) where g = c%4 (dims [256g, 256g+256) of the 1024 model dim).

Per-core device program (all matmul operands bf16, fp32 PSUM accumulation).
The scalar engine's exp stream (128 ops x ~1.04us over the 16.8M attention
scores) is the throughput floor, so the whole schedule is built to keep it
dense from ~10us (first scores) to the end:
  - QT/KT projections computed transposed: QT[d, t] = Wq_g @ x_b^T (+bias; Q
    also scaled by 1/sqrt(64)), d-major weight halves DMA'd first so the
    first score tile exists as early as the DMA pipe allows. K stored packed
    [hh*64+dh, t]; score matmuls contract over a 64-partition slice. V in
    natural [token, dim] layout.
  - Attention runs as two decoupled software-pipelined streams:
      ST/exp stream: S^T tiles [128 keys, 2 heads, 512 q] -> one wide exp per
      tile on the scalar engine into a bf16 SBUF ring (EX_WINDOW tiles). The
      2-deep score PSUM ring means ST(e+2) waits exp(e), so STs are placed in
      the PE stream by a credit scheduler at ~1us spacing, interleaved inside
      projection groups (availability-greedy order over (unit, key-tile)).
      PV stream (trails by PV_LAG): flipped PV -- ctx[q, dh] += exp(S)^T.T @ V
      with q on partitions, so each matmul charges 64 output rows instead of
      512. Softmax denominators accumulate via rhs=ones [128,1] matmuls.
      PSUM accumulator banks hold 8 groups each: start/stop are asserted once
      per bank per round (start zeroes the whole 2KB zero region).
  - Unit epilogue: one PSUM->SBUF copy releases the ctx bank for the next
    unit (~1us earlier), then DVE reciprocal + per-(head,qt) scales, PE
    transpose (via identity) back to ctx^T[(hh,dh), q]; out-projection groups
    drip into the PE gaps at unit seams; the final unit runs a per-qt
    pipelined tail with evictions split across scalar+vector engines.
  - Out partials stored bf16; host sums the 4 group partials per batch and
    adds b_out + b_v @ W_out^T (V-bias commutes through softmax).
"""

import numpy as np
import ml_dtypes

import concourse.bacc as bacc
import concourse.mybir as mybir
from concourse.tile import TileContext
from concourse.bass_utils import run_bass_kernel_spmd
from concourse.masks import make_identity

AF = mybir.ActivationFunctionType
ALU = mybir.AluOpType
F32 = mybir.dt.float32
BF16 = mybir.dt.bfloat16
NP_BF16 = ml_dtypes.bfloat16

B, S, D, H, DH = 2, 2048, 1024, 16, 64
DG = 256          # dims per head-group (4 heads)
TC = 512          # token / query chunk
NTC = S // TC     # 4
NTT = S // 128    # 16 token tiles
NKT = S // 128    # 16 key tiles

EX_WINDOW = 48    # bf16 S^T-exp tiles buffered ahead of the PV stream
PV_LAG = 4        # PV stream trails the ST/exp stream by >= this many steps
N_WARM = 6        # PE p-state warm-up matmuls

# bit pattern of two bf16 1.0s packed in a float32 (for memset on bf16 tiles)
_ONES_BF16_F32 = float(
    np.frombuffer(np.array([0x3F803F80], np.uint32).tobytes(), np.float32)[0]
)

_NC_CACHE = None


def _build_nc():
    nc = bacc.Bacc("TRN2", target_bir_lowering=False, debug=False)

    xT = nc.dram_tensor("xT", [D, S], BF16, kind="ExternalInput")
    wq0 = nc.dram_tensor("wq0", [128, 8, 128], BF16, kind="ExternalInput")
    wq1 = nc.dram_tensor("wq1", [128, 8, 128], BF16, kind="ExternalInput")
    wk0 = nc.dram_tensor("wk0", [128, 8, 128], BF16, kind="ExternalInput")
    wk1 = nc.dram_tensor("wk1", [128, 8, 128], BF16, kind="ExternalInput")
    wv = nc.dram_tensor("wvT", [D, DG], BF16, kind="ExternalInput")
    wo = nc.dram_tensor("woT", [DG, D], BF16, kind="ExternalInput")
    bq = nc.dram_tensor("bq", [2, 128], F32, kind="ExternalInput")
    bk = nc.dram_tensor("bk", [2, 128], F32, kind="ExternalInput")
    out = nc.dram_tensor("out", [S, D], BF16, kind="ExternalOutput")

    with TileContext(nc) as tc:
        with (
            tc.tile_pool(name="const", bufs=1) as constp,
            tc.tile_pool(name="xt", bufs=3) as xtp,
            tc.tile_pool(name="expst", bufs=EX_WINDOW) as expp,
            tc.tile_pool(name="ctxn", bufs=2) as ctxnp,
            tc.tile_pool(name="small", bufs=2) as smallp,
            tc.tile_pool(name="outp", bufs=4) as outp,
            tc.tile_pool(name="pp_ps", bufs=2, space="PSUM") as pp,
            tc.tile_pool(name="s_ps", bufs=2, space="PSUM") as sps,
            tc.tile_pool(name="ctx_ps", bufs=1, space="PSUM") as ctxps,
            tc.tile_pool(name="dn_ps", bufs=1, space="PSUM") as dnp,
        ):
            # ---- persistent tiles ----
            # Q/K weights are d-major: the d0 halves arrive first so the
            # first Q/K projection groups (which gate the scalar engine's
            # entire exp stream) start as early as the DMA pipe allows
            wq_s = constp.tile([128, 2, 8, 128], BF16)
            wk_s = constp.tile([128, 2, 8, 128], BF16)
            wv_s = constp.tile([128, 8, DG], BF16)
            bq_s = constp.tile([128, 2], F32)
            bk_s = constp.tile([128, 2], F32)
            xTr = xT[:, :].rearrange("(k p) t -> p k t", p=128)
            xt0 = xtp.tile([128, 8, TC], BF16, name="xt0", tag="xt")
            nc.sync.dma_start(out=wq_s[:, 0, :, :], in_=wq0[:, :, :])
            nc.sync.dma_start(out=xt0[:, 0:4, :], in_=xTr[:, 0:4, 0:TC])
            nc.sync.dma_start(out=wk_s[:, 0, :, :], in_=wk0[:, :, :])
            nc.sync.dma_start(out=xt0[:, 4:8, :], in_=xTr[:, 4:8, 0:TC])
            nc.sync.dma_start(out=bq_s, in_=bq[:, :].rearrange("t p -> p t"))
            nc.sync.dma_start(out=bk_s, in_=bk[:, :].rearrange("t p -> p t"))
            nc.sync.dma_start(out=wq_s[:, 1, :, :], in_=wq1[:, :, :])
            nc.sync.dma_start(out=wk_s[:, 1, :, :], in_=wk1[:, :, :])
            nc.sync.dma_start(out=wv_s,
                              in_=wv[:, :].rearrange("(k p) m -> p k m", p=128))

            QT_s = constp.tile([128, 2, S], BF16)
            KT_s = constp.tile([128, 2, S], BF16)
            ctxT_s = constp.tile([128, 2, S], BF16)
            Vg_s = constp.tile([128, NTT, 4, DH], BF16)
            wo_s = constp.tile([128, 2, D], BF16)
            ident = constp.tile([128, 128], BF16)
            make_identity(nc, ident)
            ones = constp.tile([128, 2], BF16)
            nc.vector.memset(ones.bitcast(F32), _ONES_BF16_F32)

            # warm the PE clock (p-state ramp) with dummy matmuls while the
            # initial DMAs stream in, so real matmuls start at full rate
            warm = constp.tile([128, TC], BF16)
            nc.vector.memset(warm.bitcast(F32), _ONES_BF16_F32)
            wps = pp.tile([128, TC], F32, tag="p", name="wps")
            for _ in range(N_WARM):
                nc.tensor.matmul(wps, lhsT=warm[:, 0:128], rhs=warm,
                                 start=True, stop=True)

            # ---- projection group emitters (q/k transposed, v natural) ----
            # note_pe is a drip hook called after each matmul with its PE cost
            def emit_q_group(d, tci, xt, note_pe):
                tsl = slice(tci * TC, (tci + 1) * TC)
                dsl = slice(d * 128, (d + 1) * 128)
                psq = pp.tile([128, TC], F32, tag="p", name="psq")
                for k in range(8):
                    nc.tensor.matmul(psq, lhsT=wq_s[:, d, k, :], rhs=xt[:, k, :],
                                     start=(k == 0), stop=(k == 7))
                    note_pe(213)
                nc.vector.tensor_scalar(QT_s[:, d, tsl], psq,
                                        scalar1=bq_s[:, d:d + 1], scalar2=0.125,
                                        op0=ALU.add, op1=ALU.mult)

            def emit_k_group(d, tci, xt, note_pe):
                tsl = slice(tci * TC, (tci + 1) * TC)
                dsl = slice(d * 128, (d + 1) * 128)
                psk = pp.tile([128, TC], F32, tag="p", name="psk")
                for k in range(8):
                    nc.tensor.matmul(psk, lhsT=wk_s[:, d, k, :], rhs=xt[:, k, :],
                                     start=(k == 0), stop=(k == 7))
                    note_pe(213)
                nc.vector.tensor_scalar(KT_s[:, d, tsl], psk,
                                        scalar1=bk_s[:, d:d + 1],
                                        scalar2=None, op0=ALU.add)

            def emit_v_group(tci, tt, xt, note_pe):
                ti = tci * 4 + tt
                psv = pp.tile([128, DG], F32, tag="p", name="psv")
                for k in range(8):
                    nc.tensor.matmul(psv, lhsT=xt[:, k, tt * 128:(tt + 1) * 128],
                                     rhs=wv_s[:, k, :], start=(k == 0), stop=(k == 7))
                    note_pe(107)
                nc.vector.tensor_copy(Vg_s[:, ti, :, :], psv)
                ti_done.add(ti)

            # ---- attention streams ----
            # units u = (qc, d); steps (u, kt). The ST/exp stream leads (bf16
            # ex ring), the PV stream trails and accumulates flipped ctx.
            units = [(qc, d) for qc in range(NTC) for d in range(2)]
            n_steps = len(units) * NKT
            ex_of = {}
            ex_idx = {}       # (u, kt) -> ST emission counter at emission
            emitted = set()
            cps_of = {}
            dn_of = {}
            drip = []
            pending_tr = []   # deferred (ready_at_pv, u, ctx_n) transposes
            q_done = set()    # (qc, d) Q groups evicted
            k_done = set()    # (d, kchunk) K groups evicted
            ti_done = set()   # V token-tiles evicted

            def emit_st(u, kt):
                qc, d = units[u]
                qsl = slice(qc * TC, (qc + 1) * TC)
                ksl = slice(kt * 128, (kt + 1) * 128)
                sp = sps.tile([128, 2, TC], F32, tag="s")
                for hh in range(2):
                    p0 = 64 * hh
                    nc.tensor.matmul(sp[:, hh, :],
                                     lhsT=KT_s[p0:p0 + 64, d, ksl],
                                     rhs=QT_s[p0:p0 + 64, d, qsl],
                                     start=True, stop=True)
                ex = expp.tile([128, 2, TC], BF16)
                nc.scalar.activation(ex, sp, AF.Exp)
                ex_of[(u, kt)] = ex

            def next_st():
                # earliest (u, kt) in PV consumption order whose Q chunk and
                # K chunk are both evicted -- front-fills the PV frontier
                for u in range(len(units)):
                    qc, d = units[u]
                    if (qc, d) not in q_done:
                        continue
                    for kt in range(NKT):
                        if (u, kt) not in emitted and (d, kt // 4) in k_done:
                            return (u, kt)
                return None

            def emit_pv(i):
                u, kt = divmod(i, NKT)
                qc, d = units[u]
                ex = ex_of.pop((u, kt))
                if kt == 0:
                    cps_of[u] = ctxps.tile([128, 8, DH], F32, tag="ctx",
                                           name=f"cps{u}")
                    dn_of[u] = dnp.tile([128, 8], F32, tag="dn", name=f"dn{u}")
                cps, dn = cps_of[u], dn_of[u]
                first, last = (kt == 0), (kt == NKT - 1)
                stk = tc.high_priority(offset=PV_PRIO) if PV_PRIO else None
                if stk:
                    stk.__enter__()
                # PSUM "start" zeroes the whole 2KB bank (zero region), so a
                # bank holding 8 accumulation groups must be started exactly
                # once (first group's first matmul) and stopped once (last
                # group's last matmul); later groups' first writes land on
                # pending-zero bytes and start cleanly.
                for hh in range(2):
                    h = 2 * d + hh
                    for qt in range(4):
                        j = hh * 4 + qt
                        lhs = ex[:, hh, qt * 128:(qt + 1) * 128]
                        nc.tensor.matmul(cps[:, j, :], lhsT=lhs,
                                         rhs=Vg_s[:, kt, h, :],
                                         start=(first and j == 0),
                                         stop=(last and j == 7))
                        nc.tensor.matmul(dn[:, j:j + 1], lhsT=lhs,
                                         rhs=ones[:, 0:1],
                                         start=(first and j == 0),
                                         stop=(last and j == 7))
                if stk:
                    stk.__exit__(None, None, None)
                if last and u == len(units) - 1:
                    emit_last_unit_tail(u)
                elif last:
                    # one fast PSUM->SBUF copy releases the ctx bank (~0.7us)
                    # so the next unit's first PV isn't stalled behind the 8
                    # normalize muls (~1.7us); the muls then read the copy
                    cps, dn = cps_of.pop(u), dn_of.pop(u)
                    rec = smallp.tile([128, 8], F32, tag="rec")
                    nc.vector.reciprocal(rec, dn)
                    ctxf = ctxnp.tile([128, 8, DH], F32, tag="ctxf")
                    nc.vector.tensor_copy(ctxf, cps)
                    ctx_n = ctxnp.tile([128, 4, 2, DH], BF16)
                    for qt in range(4):
                        for hh in range(2):
                            j = hh * 4 + qt
                            nc.vector.tensor_scalar(ctx_n[:, qt, hh, :],
                                                    ctxf[:, j, :],
                                                    scalar1=rec[:, j:j + 1],
                                                    scalar2=None, op0=ALU.mult)
                    pending_tr.append((pv_i + 2, u, ctx_n))

            def emit_transpose_stage(u, ctx_n):
                qc, d = units[u]
                psT = pp.tile([128, 4, 128], BF16, tag="p", name="psT")
                qsl = slice(qc * TC, (qc + 1) * TC)
                for qt in range(4):
                    nc.tensor.transpose(psT[:, qt, :], ctx_n[:, qt, :, :], ident)
                nc.vector.tensor_copy(ctxT_s[:, d, qsl], psT)
                if d == 1:
                    drip.extend(range(qc * 4, (qc + 1) * 4))

            def emit_last_unit_tail(u):
                # tail: per-qt chains with the DVE spine front-loaded
                # (rec + both muls per qt, transpose, psT evict, then that
                # qt's out-projection immediately); output evictions split
                # between the now-idle scalar engine and DVE
                qc, d = units[u]
                cps, dn = cps_of.pop(u), dn_of.pop(u)
                rec = smallp.tile([128, 8], F32, tag="rec")
                nc.vector.reciprocal(rec, dn)
                ctx_n = ctxnp.tile([128, 4, 2, DH], BF16)
                for qt in range(4):
                    for hh in range(2):
                        j = hh * 4 + qt
                        nc.vector.tensor_scalar(ctx_n[:, qt, hh, :],
                                                cps[:, j, :],
                                                scalar1=rec[:, j:j + 1],
                                                scalar2=None, op0=ALU.mult)
                    psT = pp.tile([128, 128], BF16, tag="p", name="psTq")
                    nc.tensor.transpose(psT, ctx_n[:, qt, :, :], ident)
                    tt = qc * 4 + qt
                    nc.vector.tensor_copy(
                        ctxT_s[:, d, tt * 128:(tt + 1) * 128], psT)
                    ev = (nc.scalar, nc.scalar) if qt == 0 else \
                        (nc.scalar, nc.vector)
                    emit_d_group(tt, evict_engines=ev,
                                 po_tag=("s", "p")[qt % 2],
                                 split_store=(qt == 3))

            def emit_d_group(tt, evict_engines=None, po_tag="p",
                             split_store=False):
                # one token-tile: both 512-wide halves of the out row block,
                # merged into a single bf16 store (split per-half for the
                # final tile so its first DMA overlaps the second eviction)
                psl = slice(tt * 128, (tt + 1) * 128)
                ot = outp.tile([128, 2, TC], BF16)
                for oc in range(2):
                    osl = slice(oc * TC, (oc + 1) * TC)
                    pool = pp if po_tag == "p" else sps
                    po = pool.tile([128, TC], F32, tag=po_tag, name="po")
                    for dd in range(2):
                        nc.tensor.matmul(po, lhsT=ctxT_s[:, dd, psl],
                                         rhs=wo_s[:, dd, osl],
                                         start=(dd == 0), stop=(dd == 1))
                    eng = (evict_engines or (nc.vector, nc.vector))[oc]
                    if eng is nc.scalar:
                        eng.copy(ot[:, oc, :], po)
                    else:
                        eng.tensor_copy(ot[:, oc, :], po)
                    if split_store:
                        nc.sync.dma_start(out=out[psl, osl], in_=ot[:, oc, :])
                if not split_store:
                    nc.sync.dma_start(out=out[psl, :], in_=ot)

            # ---- phase B: projections with interleaved ST/exp/PV streams ----
            # Credit-based drip: the score PSUM ring is only 2 deep, so
            # ST(e+2) waits for exp(e); STs must be spaced ~1.04us apart in
            # the PE stream (the exp duration) or they head-of-line block it.
            # Each emitted PE instruction adds its cost to `credit`; an ST is
            # dripped once enough other work sits between it and the last one.
            st_n = 0          # STs emitted
            pv_i = 0          # PV steps emitted (consumption order)
            credit = 10 ** 9

            def pv_ready():
                key = divmod(pv_i, NKT)
                if key not in ex_of or key[1] not in ti_done:
                    return False
                # PV must trail its exp far enough not to block the PE stream
                return ex_idx[key] <= st_n - PV_LAG or st_n == n_steps

            def note_pe(ns):
                nonlocal credit, st_n, pv_i
                credit += ns
                progress = True
                while progress:
                    progress = False
                    nxt = next_st()
                    if credit >= 500 and nxt is not None \
                            and st_n - pv_i < EX_WINDOW - 2:
                        emit_st(*nxt)
                        emitted.add(nxt)
                        ex_idx[nxt] = st_n
                        st_n += 1
                        credit = 0
                        progress = True
                    if pv_i < n_steps and pv_ready():
                        emit_pv(pv_i)
                        pv_i += 1
                        credit += 240
                        progress = True
                    if pending_tr and pending_tr[0][0] <= pv_i:
                        _, uu, cn = pending_tr.pop(0)
                        emit_transpose_stage(uu, cn)
                        credit += 213
                        progress = True

            # flat projection order: chunk 1's K groups are pulled between
            # chunk 0's V groups so the ST stream never runs out of
            # availability while the scalar engine still has backlog
            xts = [xt0]
            for tci in range(1, NTC):
                xt = xtp.tile([128, 8, TC], BF16, tag="xt", name=f"xt{tci}")
                nc.sync.dma_start(out=xt, in_=xTr[:, :, tci * TC:(tci + 1) * TC])
                if tci == 1:
                    nc.sync.dma_start(
                        out=wo_s, in_=wo[:, :].rearrange("(k p) m -> p k m", p=128))
                xts.append(xt)

            # chunk 0's first Q/K groups interleave their matmul halves
            # with the DMA half-arrivals (wq_a, xt_a, wk_a, wq_b, ...)
            psq0 = pp.tile([128, TC], F32, tag="p", name="psq0")
            psk0 = pp.tile([128, TC], F32, tag="p", name="psk0")
            for k in range(4):
                nc.tensor.matmul(psq0, lhsT=wq_s[:, 0, k, :], rhs=xt0[:, k, :],
                                 start=(k == 0), stop=False)
            for k in range(4):
                nc.tensor.matmul(psk0, lhsT=wk_s[:, 0, k, :], rhs=xt0[:, k, :],
                                 start=(k == 0), stop=False)
            for k in range(4, 8):
                nc.tensor.matmul(psq0, lhsT=wq_s[:, 0, k, :], rhs=xt0[:, k, :],
                                 start=False, stop=(k == 7))
            for k in range(4, 8):
                nc.tensor.matmul(psk0, lhsT=wk_s[:, 0, k, :], rhs=xt0[:, k, :],
                                 start=False, stop=(k == 7))
            nc.vector.tensor_scalar(QT_s[:, 0, 0:TC], psq0,
                                    scalar1=bq_s[:, 0:1], scalar2=0.125,
                                    op0=ALU.add, op1=ALU.mult)
            nc.vector.tensor_scalar(KT_s[:, 0, 0:TC], psk0,
                                    scalar1=bk_s[:, 0:1],
                                    scalar2=None, op0=ALU.add)
            q_done.add((0, 0))
            k_done.add((0, 0))
            # seed the exp stream with two back-to-back STs so exp#1 follows
            # exp#0 without a gap (the credit drip would space them apart)
            for _ in range(2):
                nxt = next_st()
                emit_st(*nxt)
                emitted.add(nxt)
                ex_idx[nxt] = st_n
                st_n += 1
            note_pe(0)

            flat = ["0q1", "0k1", "0v0", "1k0", "0v1", "1k1",
                    "0v2", "1q0", "0v3", "1q1", "1v0", "1v1", "1v2", "1v3",
                    "2k0", "2k1", "2q0", "2q1", "2v0", "2v1", "2v2", "2v3",
                    "3k0", "3k1", "3q0", "3q1", "3v0", "3v1", "3v2", "3v3"]
            for g in flat:
                tci, kind, idx = int(g[0]), g[1], int(g[2])
                if kind == "q":
                    emit_q_group(idx, tci, xts[tci], note_pe)
                    q_done.add((tci, idx))
                elif kind == "k":
                    emit_k_group(idx, tci, xts[tci], note_pe)
                    k_done.add((idx, tci))
                else:
                    emit_v_group(tci, idx, xts[tci], note_pe)

            # ---- phase C: attention steady state ----
            # PVs are emitted before STs: a PV's exp is PV_LAG behind and
            # never blocks the in-order PE stream, while an ST waits on the
            # 2-deep score ring (exp(e-2)); this drains the PV backlog faster
            # at the tail
            while pv_i < n_steps:
                did = False
                if pv_ready():
                    emit_pv(pv_i)
                    pv_i += 1
                    did = True
                nxt = next_st()
                if nxt is not None and st_n - pv_i < EX_WINDOW:
                    emit_st(*nxt)
                    emitted.add(nxt)
                    ex_idx[nxt] = st_n
                    st_n += 1
                    did = True
                if not did and pv_i < n_steps:
                    emit_pv(pv_i)
                    pv_i += 1
                if drip and (pv_i % NKT == NKT - 1 or pv_i % 3 == 0
                             or (6 * NKT <= pv_i < n_steps - 8 and
                                 pv_i % 2 == 0)):
                    # out-projection groups preferentially fill the PE bubble
                    # at unit seams; fully drained before the last unit's
                    # final steps so they don't delay the critical tail
                    emit_d_group(drip.pop(0))
                while pending_tr and pending_tr[0][0] <= pv_i:
                    _, uu, cn = pending_tr.pop(0)
                    emit_transpose_stage(uu, cn)
            # tail: alternate evictions across idle engines, po rings from
            # both free PSUM pools to avoid eviction-serialized stalls
            engs = [(nc.gpsimd, nc.vector), (nc.scalar, nc.gpsimd),
                    (nc.vector, nc.scalar)]
            for i, g in enumerate(drip):
                emit_d_group(g, evict_engines=engs[i % 3],
                             po_tag=("p", "s")[i % 2])

    nc.finalize()
    return nc


def get_nc():
    global _NC_CACHE
    if _NC_CACHE is None:
        _NC_CACHE = _build_nc()
    return _NC_CACHE


def make_in_maps(x, W_q, b_q, W_k, b_k, W_v, b_v, W_out, b_out):
    xb = [np.ascontiguousarray(x[b].T).astype(NP_BF16) for b in range(B)]
    in_maps = []
    for c in range(8):
        b, g = divmod(c, 4)
        sl = slice(DG * g, DG * (g + 1))
        wqT = W_q[sl, :].T.astype(NP_BF16)
        wkT = W_k[sl, :].T.astype(NP_BF16)

        def dmajor(wT, dd):
            # [D, 128] slice -> [128 part, 8 kchunk, 128 dim], row-contiguous
            m = wT[:, dd * 128:(dd + 1) * 128].reshape(8, 128, 128)
            return np.ascontiguousarray(m.transpose(1, 0, 2))

        in_maps.append({
            "xT": xb[b],
            "wq0": dmajor(wqT, 0), "wq1": dmajor(wqT, 1),
            "wk0": dmajor(wkT, 0), "wk1": dmajor(wkT, 1),
            "wvT": np.ascontiguousarray(W_v[sl, :].T).astype(NP_BF16),
            "woT": np.ascontiguousarray(W_out[:, sl].T).astype(NP_BF16),
            "bq": b_q[sl].reshape(2, 128).astype(np.float32),
            "bk": b_k[sl].reshape(2, 128).astype(np.float32),
        })
    return in_maps


def combine_outputs(outs, W_out, b_out, b_v):
    host_bias = (b_out + b_v @ W_out.T).astype(np.float32)
    y = np.empty((B, S, D), np.float32)
    for b in range(B):
        y[b] = (outs[4 * b].astype(np.float32)
                + outs[4 * b + 1].astype(np.float32)
                + outs[4 * b + 2].astype(np.float32)
                + outs[4 * b + 3].astype(np.float32))
        y[b] += host_bias
    return y


def kernel(x, W_q, b_q, W_k, b_k, W_v, b_v, W_out, b_out):
    x = np.asarray(x, dtype=np.float32)
    args = [np.asarray(a, dtype=np.float32)
            for a in (W_q, b_q, W_k, b_k, W_v, b_v, W_out, b_out)]
    W_q, b_q, W_k, b_k, W_v, b_v, W_out, b_out = args
    nc = get_nc()
    in_maps = make_in_maps(x, W_q, b_q, W_k, b_k, W_v, b_v, W_out, b_out)
    last_err = None
    for attempt in range(3):
        try:
            res = run_bass_kernel_spmd(nc, in_maps, core_ids=list(range(8)))
            break
        except Exception as e:  # transient device-unrecoverable flakes
            last_err = e
            import time
            time.sleep(10)
    else:
        raise last_err
    outs = [r["out"] for r in res.results]
    return combine_outputs(outs, W_out, b_out, b_v)


# revision 78
# speedup vs baseline: 1.0066x; 1.0010x over previous
"""Multi-head self-attention Trainium2 kernel.

Sharding: 8 cores = 2 batches x 4 head-groups. Core c handles batch c//4 and
heads [4g, 4g+4) where g = c%4 (dims [256g, 256g+256) of the 1024 model dim).

Per-core device program (all matmul operands bf16, fp32 PSUM accumulation).
The scalar engine's exp stream (128 ops x ~1.04us over the 16.8M attention
scores) is the throughput floor, so the whole schedule is built to keep it
dense from ~10us (first scores) to the end:
  - QT/KT projections computed transposed: QT[d, t] = Wq_g @ x_b^T (+bias; Q
    also scaled by 1/sqrt(64)), d-major weight halves DMA'd first so the
    first score tile exists as early as the DMA pipe allows. K stored packed
    [hh*64+dh, t]; score matmuls contract over a 64-partition slice. V in
    natural [token, dim] layout.
  - Attention runs as two decoupled software-pipelined streams:
      ST/exp stream: S^T tiles [128 keys, 2 heads, 512 q] -> one wide exp per
      tile on the scalar engine into a bf16 SBUF ring (EX_WINDOW tiles). The
      2-deep score PSUM ring means ST(e+2) waits exp(e), so STs are placed in
      the PE stream by a credit scheduler at ~1us spacing, interleaved inside
      projection groups (availability-greedy order over (unit, key-tile)).
      PV stream (trails by PV_LAG): flipped PV -- ctx[q, dh] += exp(S)^T.T @ V
      with q on partitions, so each matmul charges 64 output rows instead of
      512. Softmax denominators accumulate via rhs=ones [128,1] matmuls.
      PSUM accumulator banks hold 8 groups each: start/stop are asserted once
      per bank per round (start zeroes the whole 2KB zero region).
  - Unit epilogue: one PSUM->SBUF copy releases the ctx bank for the next
    unit (~1us earlier), then DVE reciprocal + per-(head,qt) scales, PE
    transpose (via identity) back to ctx^T[(hh,dh), q]; out-projection groups
    drip into the PE gaps at unit seams; the final unit runs a per-qt
    pipelined tail with evictions split across scalar+vector engines.
  - Out partials stored bf16; host sums the 4 group partials per batch and
    adds b_out + b_v @ W_out^T (V-bias commutes through softmax).
"""

import numpy as np
import ml_dtypes

import concourse.bacc as bacc
import concourse.mybir as mybir
from concourse.tile import TileContext
from concourse.bass_utils import run_bass_kernel_spmd
from concourse.masks import make_identity

AF = mybir.ActivationFunctionType
ALU = mybir.AluOpType
F32 = mybir.dt.float32
BF16 = mybir.dt.bfloat16
NP_BF16 = ml_dtypes.bfloat16

B, S, D, H, DH = 2, 2048, 1024, 16, 64
DG = 256          # dims per head-group (4 heads)
TC = 512          # token / query chunk
NTC = S // TC     # 4
NTT = S // 128    # 16 token tiles
NKT = S // 128    # 16 key tiles

EX_WINDOW = 52    # bf16 S^T-exp tiles buffered ahead of the PV stream
PV_LAG = 4        # PV stream trails the ST/exp stream by >= this many steps
N_WARM = 6        # PE p-state warm-up matmuls

# bit pattern of two bf16 1.0s packed in a float32 (for memset on bf16 tiles)
_ONES_BF16_F32 = float(
    np.frombuffer(np.array([0x3F803F80], np.uint32).tobytes(), np.float32)[0]
)

_NC_CACHE = None


def _build_nc():
    nc = bacc.Bacc("TRN2", target_bir_lowering=False, debug=False)

    xT = nc.dram_tensor("xT", [D, S], BF16, kind="ExternalInput")
    wq0 = nc.dram_tensor("wq0", [128, 8, 128], BF16, kind="ExternalInput")
    wq1 = nc.dram_tensor("wq1", [128, 8, 128], BF16, kind="ExternalInput")
    wk0 = nc.dram_tensor("wk0", [128, 8, 128], BF16, kind="ExternalInput")
    wk1 = nc.dram_tensor("wk1", [128, 8, 128], BF16, kind="ExternalInput")
    wv = nc.dram_tensor("wvT", [D, DG], BF16, kind="ExternalInput")
    wo = nc.dram_tensor("woT", [DG, D], BF16, kind="ExternalInput")
    bq = nc.dram_tensor("bq", [2, 128], F32, kind="ExternalInput")
    bk = nc.dram_tensor("bk", [2, 128], F32, kind="ExternalInput")
    out = nc.dram_tensor("out", [S, D], BF16, kind="ExternalOutput")

    with TileContext(nc) as tc:
        with (
            tc.tile_pool(name="const", bufs=1) as constp,
            tc.tile_pool(name="xt", bufs=3) as xtp,
            tc.tile_pool(name="expst", bufs=EX_WINDOW) as expp,
            tc.tile_pool(name="ctxn", bufs=2) as ctxnp,
            tc.tile_pool(name="small", bufs=2) as smallp,
            tc.tile_pool(name="outp", bufs=4) as outp,
            tc.tile_pool(name="pp_ps", bufs=2, space="PSUM") as pp,
            tc.tile_pool(name="s_ps", bufs=2, space="PSUM") as sps,
            tc.tile_pool(name="ctx_ps", bufs=1, space="PSUM") as ctxps,
            tc.tile_pool(name="dn_ps", bufs=1, space="PSUM") as dnp,
        ):
            # ---- persistent tiles ----
            # Q/K weights are d-major: the d0 halves arrive first so the
            # first Q/K projection groups (which gate the scalar engine's
            # entire exp stream) start as early as the DMA pipe allows
            wq_s = constp.tile([128, 2, 8, 128], BF16)
            wk_s = constp.tile([128, 2, 8, 128], BF16)
            wv_s = constp.tile([128, 8, DG], BF16)
            bq_s = constp.tile([128, 2], F32)
            bk_s = constp.tile([128, 2], F32)
            xTr = xT[:, :].rearrange("(k p) t -> p k t", p=128)
            xt0 = xtp.tile([128, 8, TC], BF16, name="xt0", tag="xt")
            nc.sync.dma_start(out=wq_s[:, 0, :, :], in_=wq0[:, :, :])
            nc.sync.dma_start(out=xt0[:, 0:2, :], in_=xTr[:, 0:2, 0:TC])
            nc.sync.dma_start(out=xt0[:, 2:4, :], in_=xTr[:, 2:4, 0:TC])
            nc.sync.dma_start(out=wk_s[:, 0, :, :], in_=wk0[:, :, :])
            nc.sync.dma_start(out=xt0[:, 4:6, :], in_=xTr[:, 4:6, 0:TC])
            nc.sync.dma_start(out=xt0[:, 6:8, :], in_=xTr[:, 6:8, 0:TC])
            nc.sync.dma_start(out=bq_s, in_=bq[:, :].rearrange("t p -> p t"))
            nc.sync.dma_start(out=bk_s, in_=bk[:, :].rearrange("t p -> p t"))
            nc.sync.dma_start(out=wq_s[:, 1, :, :], in_=wq1[:, :, :])
            nc.sync.dma_start(out=wk_s[:, 1, :, :], in_=wk1[:, :, :])
            nc.sync.dma_start(out=wv_s,
                              in_=wv[:, :].rearrange("(k p) m -> p k m", p=128))

            QT_s = constp.tile([128, 2, S], BF16)
            KT_s = constp.tile([128, 2, S], BF16)
            ctxT_s = constp.tile([128, 2, S], BF16)
            Vg_s = constp.tile([128, NTT, 4, DH], BF16)
            wo_s = constp.tile([128, 2, D], BF16)
            ident = constp.tile([128, 128], BF16)
            make_identity(nc, ident)
            ones = constp.tile([128, 2], BF16)
            nc.vector.memset(ones.bitcast(F32), _ONES_BF16_F32)

            # warm the PE clock (p-state ramp) with dummy matmuls while the
            # initial DMAs stream in, so real matmuls start at full rate
            warm = constp.tile([128, TC], BF16)
            nc.vector.memset(warm.bitcast(F32), _ONES_BF16_F32)
            wps = pp.tile([128, TC], F32, tag="p", name="wps")
            for _ in range(N_WARM):
                nc.tensor.matmul(wps, lhsT=warm[:, 0:128], rhs=warm,
                                 start=True, stop=True)

            # ---- projection group emitters (q/k transposed, v natural) ----
            # note_pe is a drip hook called after each matmul with its PE cost
            def emit_q_group(d, tci, xt, note_pe):
                tsl = slice(tci * TC, (tci + 1) * TC)
                dsl = slice(d * 128, (d + 1) * 128)
                psq = pp.tile([128, TC], F32, tag="p", name="psq")
                for k in range(8):
                    nc.tensor.matmul(psq, lhsT=wq_s[:, d, k, :], rhs=xt[:, k, :],
                                     start=(k == 0), stop=(k == 7))
                    note_pe(213)
                nc.vector.tensor_scalar(QT_s[:, d, tsl], psq,
                                        scalar1=bq_s[:, d:d + 1], scalar2=0.125,
                                        op0=ALU.add, op1=ALU.mult)

            def emit_k_group(d, tci, xt, note_pe):
                tsl = slice(tci * TC, (tci + 1) * TC)
                dsl = slice(d * 128, (d + 1) * 128)
                psk = pp.tile([128, TC], F32, tag="p", name="psk")
                for k in range(8):
                    nc.tensor.matmul(psk, lhsT=wk_s[:, d, k, :], rhs=xt[:, k, :],
                                     start=(k == 0), stop=(k == 7))
                    note_pe(213)
                nc.vector.tensor_scalar(KT_s[:, d, tsl], psk,
                                        scalar1=bk_s[:, d:d + 1],
                                        scalar2=None, op0=ALU.add)

            def emit_v_group(tci, tt, xt, note_pe):
                ti = tci * 4 + tt
                psv = pp.tile([128, DG], F32, tag="p", name="psv")
                for k in range(8):
                    nc.tensor.matmul(psv, lhsT=xt[:, k, tt * 128:(tt + 1) * 128],
                                     rhs=wv_s[:, k, :], start=(k == 0), stop=(k == 7))
                    note_pe(107)
                nc.vector.tensor_copy(Vg_s[:, ti, :, :], psv)
                ti_done.add(ti)

            # ---- attention streams ----
            # units u = (qc, d); steps (u, kt). The ST/exp stream leads (bf16
            # ex ring), the PV stream trails and accumulates flipped ctx.
            units = [(qc, d) for qc in range(NTC) for d in range(2)]
            n_steps = len(units) * NKT
            ex_of = {}
            ex_idx = {}       # (u, kt) -> ST emission counter at emission
            emitted = set()
            cps_of = {}
            dn_of = {}
            drip = []
            pending_tr = []   # deferred (ready_at_pv, u, ctx_n) transposes
            q_done = set()    # (qc, d) Q groups evicted
            k_done = set()    # (d, kchunk) K groups evicted
            ti_done = set()   # V token-tiles evicted

            def emit_st(u, kt):
                qc, d = units[u]
                qsl = slice(qc * TC, (qc + 1) * TC)
                ksl = slice(kt * 128, (kt + 1) * 128)
                sp = sps.tile([128, 2, TC], F32, tag="s")
                for hh in range(2):
                    p0 = 64 * hh
                    nc.tensor.matmul(sp[:, hh, :],
                                     lhsT=KT_s[p0:p0 + 64, d, ksl],
                                     rhs=QT_s[p0:p0 + 64, d, qsl],
                                     start=True, stop=True)
                ex = expp.tile([128, 2, TC], BF16)
                nc.scalar.activation(ex, sp, AF.Exp)
                ex_of[(u, kt)] = ex

            def next_st():
                # earliest (u, kt) in PV consumption order whose Q chunk and
                # K chunk are both evicted -- front-fills the PV frontier
                for u in range(len(units)):
                    qc, d = units[u]
                    if (qc, d) not in q_done:
                        continue
                    for kt in range(NKT):
                        if (u, kt) not in emitted and (d, kt // 4) in k_done:
                            return (u, kt)
                return None

            def emit_pv(i):
                u, kt = divmod(i, NKT)
                qc, d = units[u]
                ex = ex_of.pop((u, kt))
                if kt == 0:
                    cps_of[u] = ctxps.tile([128, 8, DH], F32, tag="ctx",
                                           name=f"cps{u}")
                    dn_of[u] = dnp.tile([128, 8], F32, tag="dn", name=f"dn{u}")
                cps, dn = cps_of[u], dn_of[u]
                first, last = (kt == 0), (kt == NKT - 1)
                stk = tc.high_priority(offset=PV_PRIO) if PV_PRIO else None
                if stk:
                    stk.__enter__()
                # PSUM "start" zeroes the whole 2KB bank (zero region), so a
                # bank holding 8 accumulation groups must be started exactly
                # once (first group's first matmul) and stopped once (last
                # group's last matmul); later groups' first writes land on
                # pending-zero bytes and start cleanly.
                for hh in range(2):
                    h = 2 * d + hh
                    for qt in range(4):
                        j = hh * 4 + qt
                        lhs = ex[:, hh, qt * 128:(qt + 1) * 128]
                        nc.tensor.matmul(cps[:, j, :], lhsT=lhs,
                                         rhs=Vg_s[:, kt, h, :],
                                         start=(first and j == 0),
                                         stop=(last and j == 7))
                        nc.tensor.matmul(dn[:, j:j + 1], lhsT=lhs,
                                         rhs=ones[:, 0:1],
                                         start=(first and j == 0),
                                         stop=(last and j == 7))
                if stk:
                    stk.__exit__(None, None, None)
                if last and u == len(units) - 1:
                    emit_last_unit_tail(u)
                elif last:
                    # one fast PSUM->SBUF copy releases the ctx bank (~0.7us)
                    # so the next unit's first PV isn't stalled behind the 8
                    # normalize muls (~1.7us); the muls then read the copy
                    cps, dn = cps_of.pop(u), dn_of.pop(u)
                    rec = smallp.tile([128, 8], F32, tag="rec")
                    nc.vector.reciprocal(rec, dn)
                    ctxf = ctxnp.tile([128, 8, DH], F32, tag="ctxf")
                    nc.vector.tensor_copy(ctxf, cps)
                    ctx_n = ctxnp.tile([128, 4, 2, DH], BF16)
                    for qt in range(4):
                        for hh in range(2):
                            j = hh * 4 + qt
                            nc.vector.tensor_scalar(ctx_n[:, qt, hh, :],
                                                    ctxf[:, j, :],
                                                    scalar1=rec[:, j:j + 1],
                                                    scalar2=None, op0=ALU.mult)
                    pending_tr.append((pv_i + 2, u, ctx_n))

            def emit_transpose_stage(u, ctx_n):
                qc, d = units[u]
                psT = pp.tile([128, 4, 128], BF16, tag="p", name="psT")
                qsl = slice(qc * TC, (qc + 1) * TC)
                for qt in range(4):
                    nc.tensor.transpose(psT[:, qt, :], ctx_n[:, qt, :, :], ident)
                nc.vector.tensor_copy(ctxT_s[:, d, qsl], psT)
                if d == 1:
                    drip.extend(range(qc * 4, (qc + 1) * 4))

            def emit_last_unit_tail(u):
                # tail: per-qt chains with the DVE spine front-loaded
                # (rec + both muls per qt, transpose, psT evict, then that
                # qt's out-projection immediately); output evictions split
                # between the now-idle scalar engine and DVE
                qc, d = units[u]
                cps, dn = cps_of.pop(u), dn_of.pop(u)
                rec = smallp.tile([128, 8], F32, tag="rec")
                nc.vector.reciprocal(rec, dn)
                ctx_n = ctxnp.tile([128, 4, 2, DH], BF16)
                for qt in range(4):
                    for hh in range(2):
                        j = hh * 4 + qt
                        nc.vector.tensor_scalar(ctx_n[:, qt, hh, :],
                                                cps[:, j, :],
                                                scalar1=rec[:, j:j + 1],
                                                scalar2=None, op0=ALU.mult)
                    psT = pp.tile([128, 128], BF16, tag="p", name="psTq")
                    nc.tensor.transpose(psT, ctx_n[:, qt, :, :], ident)
                    tt = qc * 4 + qt
                    nc.vector.tensor_copy(
                        ctxT_s[:, d, tt * 128:(tt + 1) * 128], psT)
                    ev = (nc.scalar, nc.scalar) if qt == 0 else \
                        (nc.scalar, nc.vector)
                    emit_d_group(tt, evict_engines=ev,
                                 po_tag=("s", "p")[qt % 2],
                                 split_store=(qt == 3))

            def emit_d_group(tt, evict_engines=None, po_tag="p",
                             split_store=False):
                # one token-tile: both 512-wide halves of the out row block,
                # merged into a single bf16 store (split per-half for the
                # final tile so its first DMA overlaps the second eviction)
                psl = slice(tt * 128, (tt + 1) * 128)
                ot = outp.tile([128, 2, TC], BF16)
                for oc in range(2):
                    osl = slice(oc * TC, (oc + 1) * TC)
                    pool = pp if po_tag == "p" else sps
                    po = pool.tile([128, TC], F32, tag=po_tag, name="po")
                    for dd in range(2):
                        nc.tensor.matmul(po, lhsT=ctxT_s[:, dd, psl],
                                         rhs=wo_s[:, dd, osl],
                                         start=(dd == 0), stop=(dd == 1))
                    eng = (evict_engines or (nc.vector, nc.vector))[oc]
                    if eng is nc.scalar:
                        eng.copy(ot[:, oc, :], po)
                    else:
                        eng.tensor_copy(ot[:, oc, :], po)
                    if split_store:
                        nc.sync.dma_start(out=out[psl, osl], in_=ot[:, oc, :])
                if not split_store:
                    nc.sync.dma_start(out=out[psl, :], in_=ot)

            # ---- phase B: projections with interleaved ST/exp/PV streams ----
            # Credit-based drip: the score PSUM ring is only 2 deep, so
            # ST(e+2) waits for exp(e); STs must be spaced ~1.04us apart in
            # the PE stream (the exp duration) or they head-of-line block it.
            # Each emitted PE instruction adds its cost to `credit`; an ST is
            # dripped once enough other work sits between it and the last one.
            st_n = 0          # STs emitted
            pv_i = 0          # PV steps emitted (consumption order)
            credit = 10 ** 9

            def pv_ready():
                key = divmod(pv_i, NKT)
                if key not in ex_of or key[1] not in ti_done:
                    return False
                # PV must trail its exp far enough not to block the PE stream
                return ex_idx[key] <= st_n - PV_LAG or st_n == n_steps

            def note_pe(ns):
                nonlocal credit, st_n, pv_i
                credit += ns
                progress = True
                while progress:
                    progress = False
                    nxt = next_st()
                    if credit >= 500 and nxt is not None \
                            and st_n - pv_i < EX_WINDOW - 2:
                        emit_st(*nxt)
                        emitted.add(nxt)
                        ex_idx[nxt] = st_n
                        st_n += 1
                        credit = 0
                        progress = True
                    if pv_i < n_steps and pv_ready():
                        emit_pv(pv_i)
                        pv_i += 1
                        credit += 240
                        progress = True
                    if pending_tr and pending_tr[0][0] <= pv_i:
                        _, uu, cn = pending_tr.pop(0)
                        emit_transpose_stage(uu, cn)
                        credit += 213
                        progress = True

            # flat projection order: chunk 1's K groups are pulled between
            # chunk 0's V groups so the ST stream never runs out of
            # availability while the scalar engine still has backlog
            xts = [xt0]
            for tci in range(1, NTC):
                xt = xtp.tile([128, 8, TC], BF16, tag="xt", name=f"xt{tci}")
                nc.sync.dma_start(out=xt, in_=xTr[:, :, tci * TC:(tci + 1) * TC])
                if tci == 1:
                    nc.sync.dma_start(
                        out=wo_s, in_=wo[:, :].rearrange("(k p) m -> p k m", p=128))
                xts.append(xt)

            # chunk 0's first Q/K groups interleave their matmul halves
            # with the DMA half-arrivals (wq_a, xt_a, wk_a, wq_b, ...)
            psq0 = pp.tile([128, TC], F32, tag="p", name="psq0")
            psk0 = pp.tile([128, TC], F32, tag="p", name="psk0")
            for k in range(4):
                nc.tensor.matmul(psq0, lhsT=wq_s[:, 0, k, :], rhs=xt0[:, k, :],
                                 start=(k == 0), stop=False)
            for k in range(4):
                nc.tensor.matmul(psk0, lhsT=wk_s[:, 0, k, :], rhs=xt0[:, k, :],
                                 start=(k == 0), stop=False)
            for k in range(4, 8):
                nc.tensor.matmul(psq0, lhsT=wq_s[:, 0, k, :], rhs=xt0[:, k, :],
                                 start=False, stop=(k == 7))
            for k in range(4, 8):
                nc.tensor.matmul(psk0, lhsT=wk_s[:, 0, k, :], rhs=xt0[:, k, :],
                                 start=False, stop=(k == 7))
            nc.vector.tensor_scalar(QT_s[:, 0, 0:TC], psq0,
                                    scalar1=bq_s[:, 0:1], scalar2=0.125,
                                    op0=ALU.add, op1=ALU.mult)
            nc.vector.tensor_scalar(KT_s[:, 0, 0:TC], psk0,
                                    scalar1=bk_s[:, 0:1],
                                    scalar2=None, op0=ALU.add)
            q_done.add((0, 0))
            k_done.add((0, 0))
            # seed the exp stream with two back-to-back STs so exp#1 follows
            # exp#0 without a gap (the credit drip would space them apart)
            for _ in range(2):
                nxt = next_st()
                emit_st(*nxt)
                emitted.add(nxt)
                ex_idx[nxt] = st_n
                st_n += 1
            note_pe(0)

            flat = ["0q1", "0k1", "0v0", "1k0", "0v1", "1k1",
                    "0v2", "1q0", "0v3", "1q1", "1v0", "1v1", "1v2", "1v3",
                    "2k0", "2k1", "2q0", "2q1", "2v0", "2v1", "2v2", "2v3",
                    "3k0", "3k1", "3q0", "3q1", "3v0", "3v1", "3v2", "3v3"]
            for g in flat:
                tci, kind, idx = int(g[0]), g[1], int(g[2])
                if kind == "q":
                    emit_q_group(idx, tci, xts[tci], note_pe)
                    q_done.add((tci, idx))
                elif kind == "k":
                    emit_k_group(idx, tci, xts[tci], note_pe)
                    k_done.add((idx, tci))
                else:
                    emit_v_group(tci, idx, xts[tci], note_pe)

            # ---- phase C: attention steady state ----
            # PVs are emitted before STs: a PV's exp is PV_LAG behind and
            # never blocks the in-order PE stream, while an ST waits on the
            # 2-deep score ring (exp(e-2)); this drains the PV backlog faster
            # at the tail
            while pv_i < n_steps:
                did = False
                if pv_ready():
                    emit_pv(pv_i)
                    pv_i += 1
                    did = True
                nxt = next_st()
                if nxt is not None and st_n - pv_i < EX_WINDOW:
                    emit_st(*nxt)
                    emitted.add(nxt)
                    ex_idx[nxt] = st_n
                    st_n += 1
                    did = True
                if not did and pv_i < n_steps:
                    emit_pv(pv_i)
                    pv_i += 1
                if drip and (pv_i % NKT == NKT - 1 or pv_i % 3 == 0
                             or (6 * NKT <= pv_i < n_steps - 8 and
                                 pv_i % 2 == 0)):
                    # out-projection groups preferentially fill the PE bubble
                    # at unit seams; fully drained before the last unit's
                    # final steps so they don't delay the critical tail
                    emit_d_group(drip.pop(0))
                while pending_tr and pending_tr[0][0] <= pv_i:
                    _, uu, cn = pending_tr.pop(0)
                    emit_transpose_stage(uu, cn)
            # tail: alternate evictions across idle engines, po rings from
            # both free PSUM pools to avoid eviction-serialized stalls
            engs = [(nc.gpsimd, nc.vector), (nc.scalar, nc.gpsimd),
                    (nc.vector, nc.scalar)]
            for i, g in enumerate(drip):
                emit_d_group(g, evict_engines=engs[i % 3],
                             po_tag=("p", "s")[i % 2])

    nc.finalize()
    return nc


def get_nc():
    global _NC_CACHE
    if _NC_CACHE is None:
        _NC_CACHE = _build_nc()
    return _NC_CACHE


def make_in_maps(x, W_q, b_q, W_k, b_k, W_v, b_v, W_out, b_out):
    xb = [np.ascontiguousarray(x[b].T).astype(NP_BF16) for b in range(B)]
    in_maps = []
    for c in range(8):
        b, g = divmod(c, 4)
        sl = slice(DG * g, DG * (g + 1))
        wqT = W_q[sl, :].T.astype(NP_BF16)
        wkT = W_k[sl, :].T.astype(NP_BF16)

        def dmajor(wT, dd):
            # [D, 128] slice -> [128 part, 8 kchunk, 128 dim], row-contiguous
            m = wT[:, dd * 128:(dd + 1) * 128].reshape(8, 128, 128)
            return np.ascontiguousarray(m.transpose(1, 0, 2))

        in_maps.append({
            "xT": xb[b],
            "wq0": dmajor(wqT, 0), "wq1": dmajor(wqT, 1),
            "wk0": dmajor(wkT, 0), "wk1": dmajor(wkT, 1),
            "wvT": np.ascontiguousarray(W_v[sl, :].T).astype(NP_BF16),
            "woT": np.ascontiguousarray(W_out[:, sl].T).astype(NP_BF16),
            "bq": b_q[sl].reshape(2, 128).astype(np.float32),
            "bk": b_k[sl].reshape(2, 128).astype(np.float32),
        })
    return in_maps


def combine_outputs(outs, W_out, b_out, b_v):
    host_bias = (b_out + b_v @ W_out.T).astype(np.float32)
    y = np.empty((B, S, D), np.float32)
    for b in range(B):
        y[b] = (outs[4 * b].astype(np.float32)
                + outs[4 * b + 1].astype(np.float32)
                + outs[4 * b + 2].astype(np.float32)
                + outs[4 * b + 3].astype(np.float32))
        y[b] += host_bias
    return y


def kernel(x, W_q, b_q, W_k, b_k, W_v, b_v, W_out, b_out):
    x = np.asarray(x, dtype=np.float32)
    args = [np.asarray(a, dtype=np.float32)
            for a in (W_q, b_q, W_k, b_k, W_v, b_v, W_out, b_out)]
    W_q, b_q, W_k, b_k, W_v, b_v, W_out, b_out = args
    nc = get_nc()
    in_maps = make_in_maps(x, W_q, b_q, W_k, b_k, W_v, b_v, W_out, b_out)
    last_err = None
    for attempt in range(3):
        try:
            res = run_bass_kernel_spmd(nc, in_maps, core_ids=list(range(8)))
            break
        except Exception as e:  # transient device-unrecoverable flakes
            last_err = e
            import time
            time.sleep(10)
    else:
        raise last_err
    outs = [r["out"] for r in res.results]
    return combine_outputs(outs, W_out, b_out, b_v)
